# revision 23
# baseline (speedup 1.0000x reference)
"""Hard-negative contrastive loss on 8 TRN2 NeuronCores (Bass/Tile).

Reference semantics (B=1024, Q=32, D=512, temp scalar):
    sim[i,j,q] = fusion[i] . target[j,q];  v[i,j] = max_q sim / temp
    loss = mean_i(lse_j(v[i,:]) - v[i,i])
         + 0.5 * mean_i(log(exp(pos) + sum exp(top512 offdiag)) - pos)

Sharding: target rows j are split 128/core. Each core computes its
(1024 x 128) column block of v with bf16 matmuls (contraction d on
partitions, host-side pre-transposed/scaled inputs). The jq-block loop
is outermost so after half the blocks every i-tile's first 64 columns
are done: that half is exchanged with a bf16 AllToAll that overlaps
the remaining matmuls; a second AllToAll ships the other half. Core c
then holds full rows c*128..c*128+127 and computes per-row losses in
the exp domain: E = exp(v - m) in [0, 1]; exp(pos - m) is extracted
with a one-hot accumulate so the m terms cancel (loss = ln(sum) -
ln(epos)); a 10-step bisection on per-row counts finds the top-512
threshold; an exact f32 pass sums the selected exps with a boundary
correction (remaining slots filled at the largest excluded value).
Host averages the 1024 per-row losses.
"""
import sys

if "/opt/trn_rl_repo" not in sys.path:
    sys.path.insert(0, "/opt/trn_rl_repo")

import numpy as np

N_CORES = 8
B, Q, D = 1024, 32, 512
JQ = (B // N_CORES) * Q        # 4096 target vectors per core
KC = D // 128                  # 4 contraction chunks
NBLK = 512                     # jq per matmul / psum tile
NB = JQ // NBLK                # 8 jq blocks
JBLK = NBLK // Q               # 16 j columns per psum tile
HCOL = (NB // 2) * JBLK        # 64 j columns per exchange half
N_ITERS = 8                    # bisection update steps in exp domain
NUM_HARD = B // 2              # 512

_RUNNER = None


def _build():
    import concourse.bacc as bacc
    import concourse.mybir as mybir
    import concourse.tile as tile

    f32 = mybir.dt.float32
    bf16 = mybir.dt.bfloat16
    Alu = mybir.AluOpType
    Act = mybir.ActivationFunctionType
    X = mybir.AxisListType.X

    nc = bacc.Bacc(None, target_bir_lowering=False, debug=False,
                   num_devices=N_CORES)

    # host-prepped layouts: partition dim first, contiguous per DMA chunk
    fus_ap = nc.dram_tensor("fusb", [128, KC, B], bf16, kind="ExternalInput").ap()
    tgt_ap = nc.dram_tensor("tgtb", [128, NB, KC, NBLK], bf16,
                            kind="ExternalInput").ap()
    onehi_ap = nc.dram_tensor("onehi", [128, B], f32, kind="ExternalInput").ap()
    out_ap = nc.dram_tensor("rowloss", [128, 2], f32, kind="ExternalOutput").ap()

    with tile.TileContext(nc) as tc:
        with (
            tc.tile_pool(name="fus", bufs=1) as fus_pool,
            tc.tile_pool(name="tgt", bufs=1) as tgt_pool,
            tc.tile_pool(name="res", bufs=1) as res_pool,
            tc.tile_pool(name="big", bufs=1) as big_pool,
            tc.tile_pool(name="small", bufs=1) as small_pool,
            tc.tile_pool(name="psum", bufs=1, space="PSUM") as psum_pool,
            tc.tile_pool(name="dram", bufs=1, space="DRAM") as dram_pool,
        ):
            # ---------- inputs: contiguous DMAs on parallel queues ----------
            fus = fus_pool.tile([128, KC, B], bf16)
            for k in range(KC):
                nc.sync.dma_start(fus[:, k, :], fus_ap[:, k, :])
            tgt = tgt_pool.tile([128, NB, KC, NBLK], bf16)
            for b in range(NB):
                nc.gpsimd.dma_start(tgt[:, b, :, :], tgt_ap[:, b, :, :])
            onehi = big_pool.tile([128, B], f32)
            nc.scalar.dma_start(onehi[:], onehi_ap[:])

            # ---------- phase 1: my (1024 x 128) block of v ----------
            # jq-block outer: after blocks 0..3 every i-tile's columns 0..63
            # are finished, so that half exchanges while blocks 4..7 compute
            P_sb = res_pool.tile([128, N_CORES, 128], bf16)  # [i_part, i_tile, j]
            # exchange parts: columns 0..63 after block 3, 64..95 after
            # block 5, 96..127 after block 7 — the first two collectives
            # hide behind the remaining matmul blocks
            PARTS = [(0, 64, 3), (64, 96, 5), (96, 128, 7)]
            p_in = [dram_pool.tile([B, hi - lo], bf16, name=f"p_in{i}",
                                   tag=f"p_in{i}")
                    for i, (lo, hi, _) in enumerate(PARTS)]
            p_out = [dram_pool.tile([B, hi - lo], bf16, name=f"p_out{i}",
                                    tag=f"p_out{i}")
                     for i, (lo, hi, _) in enumerate(PARTS)]

            def exchange_part(i):
                lo, hi, _ = PARTS[i]
                for it in range(N_CORES):
                    nc.sync.dma_start(
                        p_in[i][it * 128:(it + 1) * 128, :],
                        P_sb[:, it, lo:hi])
                nc.gpsimd.collective_compute(
                    "AllToAll",
                    Alu.bypass,
                    replica_groups=[list(range(N_CORES))],
                    ins=[p_in[i].opt()],
                    outs=[p_out[i].opt()],
                )

            for b in range(NB):
                ps = [psum_pool.tile([128, NBLK], f32, name=f"ps{it}", tag=f"ps{it}")
                      for it in range(N_CORES)]
                for it in range(N_CORES):
                    for k in range(KC):
                        nc.tensor.matmul(
                            ps[it][:],
                            fus[:, k, it * 128:(it + 1) * 128],
                            tgt[:, b, k, :],
                            start=(k == 0),
                            stop=(k == KC - 1),
                        )
                for it in range(N_CORES):
                    nc.vector.reduce_max(
                        P_sb[:, it, b * JBLK:(b + 1) * JBLK],
                        ps[it].rearrange("p (j q) -> p j q", q=Q),
                        axis=X,
                    )
                for i, (_, _, after_b) in enumerate(PARTS):
                    if b == after_b:
                        exchange_part(i)

            # ---------- assemble full rows ----------
            Vb = big_pool.tile([128, B], bf16)
            Vbs = Vb.rearrange("p (s j) -> p s j", s=N_CORES)
            for i, (lo, hi, _) in enumerate(PARTS):
                nc.sync.dma_start(
                    Vbs[:, :, lo:hi],
                    p_out[i].rearrange("(s p) j -> p s j", s=N_CORES))

            # ---------- phase 2: per-row losses (exp domain) ----------
            E = big_pool.tile([128, B], f32)
            Em = big_pool.tile([128, B], f32)
            junk = big_pool.tile([128, B], f32)
            junkb = big_pool.tile([128, B], bf16)

            def sm(name, dt=f32):
                return small_pool.tile([128, 1], dt, name=name, tag=name)

            m, negm, summask, epos, sumfull = (
                sm(n) for n in "m negm summask epos sumfull".split())
            mid, s01, delta, cnt, cnt_hi = (
                sm(n) for n in "mid s01 delta cnt cnt_hi".split())
            sumsel, w, rem, acc = (sm(n) for n in "sumsel w rem acc".split())

            nc.vector.reduce_max(m[:], Vb[:], axis=X)
            nc.vector.tensor_scalar_mul(negm[:], m[:], -1.0)
            # E = exp(V - m) in (0, 1], sumfull = sum E (Act accumulator);
            # Em = off-diag only; epos = diag term by subtraction
            nc.scalar.activation(E[:], Vb[:], Act.Exp, bias=negm[:], scale=1.0,
                                 accum_out=sumfull[:])
            nc.vector.scalar_tensor_tensor(
                Em[:], onehi[:], 1.0, E[:], op0=Alu.mult, op1=Alu.mult,
                accum_out=summask[:])
            nc.vector.tensor_sub(epos[:], sumfull[:], summask[:])

            # bisection for the top-512 threshold on exp values in [0, 1]:
            # mid += step * sign(cnt - 512), step halves each iteration
            nc.vector.memset(mid[:], 0.5)
            step = 0.25
            for _ in range(N_ITERS):
                nc.vector.tensor_scalar(
                    junkb[:], Em[:], mid[:], None, op0=Alu.is_gt,
                    op1=Alu.add, accum_out=cnt[:])
                nc.vector.tensor_scalar(s01[:], cnt[:], float(NUM_HARD), None,
                                        op0=Alu.is_gt)
                nc.vector.tensor_scalar(delta[:], s01[:], 2.0 * step, -step,
                                        op0=Alu.mult, op1=Alu.add)
                nc.vector.tensor_add(mid[:], mid[:], delta[:])
                step *= 0.5

            # selection at threshold mid (one count + one sum pass):
            # acc = epos + sumsel + (512-cnt_hi)*mid (excluded values ~ mid,
            # error bounded by the final bisection interval)
            nc.vector.tensor_scalar(
                junkb[:], Em[:], mid[:], None, op0=Alu.is_gt, op1=Alu.add,
                accum_out=cnt_hi[:])
            nc.vector.scalar_tensor_tensor(
                junk[:], Em[:], mid[:], Em[:], op0=Alu.is_gt, op1=Alu.mult,
                accum_out=sumsel[:])
            nc.vector.tensor_scalar(rem[:], cnt_hi[:], -1.0, float(NUM_HARD),
                                    op0=Alu.mult, op1=Alu.add)
            nc.vector.tensor_mul(rem[:], rem[:], mid[:])
            nc.vector.tensor_add(acc[:], epos[:], sumsel[:])
            nc.vector.tensor_add(acc[:], acc[:], rem[:])

            outs = res_pool.tile([128, 2], f32)
            # m cancels: loss_std = ln(sumfull) - ln(epos)
            #            loss_hard = ln(acc) - ln(epos)
            lnf, lnh, lnp = sm("lnf"), sm("lnh"), sm("lnp")
            nc.scalar.activation(lnf[:], sumfull[:], Act.Ln)
            nc.scalar.activation(lnh[:], acc[:], Act.Ln)
            nc.scalar.activation(lnp[:], epos[:], Act.Ln)
            nc.vector.tensor_sub(outs[:, 0:1], lnf[:], lnp[:])
            nc.vector.tensor_sub(outs[:, 1:2], lnh[:], lnp[:])

            nc.sync.dma_start(out_ap[:], outs[:])

    nc.compile()
    return nc


def _get_nc():
    global _RUNNER
    if _RUNNER is None:
        _RUNNER = _build()
    return _RUNNER


def make_in_maps(fusion_feats, target_feats, temp):
    import ml_dtypes

    fusion = np.asarray(fusion_feats, dtype=np.float32)
    target = np.asarray(target_feats, dtype=np.float32)
    scale = np.float32(1.0 / float(np.asarray(temp)))
    # fusb[p, k, i] = fusion[i, k*128+p] * scale
    fusb = np.ascontiguousarray(
        (fusion * scale).T.reshape(KC, 128, B).transpose(1, 0, 2)
    ).astype(ml_dtypes.bfloat16)
    rows_per = B // N_CORES
    in_maps = []
    for c in range(N_CORES):
        shard = target[c * rows_per:(c + 1) * rows_per].reshape(JQ, D)
        # tgtb[p, b, k, j] = shard[b*NBLK+j, k*128+p]
        tgtb = np.ascontiguousarray(
            shard.T.reshape(KC, 128, NB, NBLK).transpose(1, 2, 0, 3)
        ).astype(ml_dtypes.bfloat16)
        onehot = np.zeros((rows_per, B), dtype=np.float32)
        onehot[np.arange(rows_per), c * rows_per + np.arange(rows_per)] = 1.0
        in_maps.append({"fusb": fusb, "tgtb": tgtb,
                        "onehi": np.float32(1.0) - onehot})
    return in_maps


def combine(results):
    rows = np.concatenate([r["rowloss"] for r in results], axis=0)  # (1024, 2)
    loss = rows[:, 0].mean(dtype=np.float32) \
        + np.float32(0.5) * rows[:, 1].mean(dtype=np.float32)
    return np.asarray(loss, dtype=np.float32)


def kernel(fusion_feats, target_feats, temp):
    from concourse import bass_utils

    nc = _get_nc()
    in_maps = make_in_maps(fusion_feats, target_feats, temp)
    res = bass_utils.run_bass_kernel_spmd(nc, in_maps, list(range(N_CORES)))
    return combine(res.results)
